# revision 1
# baseline (speedup 1.0000x reference)
"""Trainium2 Bass kernel for nn_MoE_4088808866374.

Top-1 MoE (B=4, S=1024, D=1024, E=8, F=2816, K=1) + shared expert.
The reference computes all 8 experts densely over all 4096 tokens, but the
sigmoid gate is exactly 0 for non-top-1 experts (sigmoid(-inf)), and zero
inputs propagate exactly through SwiGLU, so a sparse dispatch computes the
identical result with ~4.5x fewer FLOPs: each token runs 2 SwiGLU passes
(shared + its top-1 expert).

Design (vs the float32r/584-capacity baseline, ~260us co-measured):
  - fp16 device dtype: 1 cyc/row on the PE like bf16 (cost model), ~8x
    better mantissa than bf16 (5.1e-4 end-to-end rel err), halves DMA
    vs float32r, and lifts f32r's moving-dim>=256 full-speed constraint.
  - y-phase transposed: stationary = w2 block [128f x 128d], moving = mid
    tokens; streams exactly ntok rows, removing the 128-token output-tile
    quantization of the baseline y-phase.
  - Expert-slot sharding: each core runs 3 segments (shared 512 tokens +
    two expert slots of 292/240). Which expert weights fill each slot is
    host-side data, so overloaded experts split across cores: per-core
    tokens = 512+292+240 = 1044 vs baseline 512+584 = 1096. A tiny DFS
    solver picks slot assignments; capacities auto-grow if counts shift.
  - 4 PSUM banks for the y-phase (2+2+4 = all 8 banks): measured large
    gain (~217->~150us in-state) - with 3 banks the PE stalls on bank
    recycling while ACT copies drain behind h-phase silus.
  - Router + dispatch + combine run host-side (measured device program is
    the expert compute, as in the baseline).

Co-measured against the baseline in one process (same chip power state):
baseline 260.6us -> this kernel ~205-230us (nrep=33 slope); absolute
numbers swing 150-350us with the chip's thermal/HAM state.
"""

import math

import numpy as np

import concourse.bacc as bacc
import concourse.mybir as mybir
import concourse.tile as tile
from concourse import bass_utils

# Problem constants (hardcoded per harness contract).
B, S, D, E, F = 4, 1024, 1024, 8, 2816
A = B * S            # 4096 tokens
T = A // E           # 512 shared-expert tokens per core
P = 128
D_CH = D // P        # 8
F_CH = F // P        # 22

# (shared, slotA, slotB) tokens per core. 2*292 >= 583 (max expert count for
# the key-0 inputs); the solver in prepare() re-derives/bumps if counts move.
SEGS_DEFAULT = (T, 292, 240)

_BUILD_CACHE = {}


def _chunks(n, cap):
    """Near-equal token chunks, each <= cap (PSUM caps a chunk at 512 fp32)."""
    k = math.ceil(n / cap)
    base = n // k
    return [base + (1 if i < n - base * k else 0) for i in range(k)]


def _build(cdt_name: str, segs: tuple, reps: int = 1,
           h_cap: int = 512, y_cap: int = 512, psy_bufs: int = 4,
           ycopy_dve: bool = False, ldw_probe: bool = False):
    """Build + compile the SPMD Bass kernel for per-core token segments.

    Each segment i has its own x, fused w1/w3, w2 and y tensors; segment 0
    is the shared expert (T tokens), segments 1.. are expert slots whose
    weight contents are chosen host-side.

    reps>1 wraps the body in a hardware For_i loop (used by the test harness
    to measure per-execution device time as a slope, amortizing the ~100ms
    axon dispatch overhead)."""
    key = (cdt_name, tuple(segs), reps, h_cap, y_cap, psy_bufs, ycopy_dve,
           ldw_probe)
    if key in _BUILD_CACHE:
        return _BUILD_CACHE[key]

    sdt = getattr(mybir.dt, cdt_name)
    fp32 = mybir.dt.float32

    nc = bacc.Bacc("TRN2", target_bir_lowering=False, debug=False)

    # DRAM I/O (per core). Host-packed layouts, contiguous per partition:
    #   x{i}:   [P(d_inner), D_CH, n]
    #   w13{i}: [P(d_inner), F_CH, 2, D_CH, P(f_inner)]  (w1|w3 per f-chunk)
    #   w2{i}:  [P(f_inner), F_CH, D]
    #   y{i}:   [D, n] fp32 (y transposed: partition dim = d)
    xs_d, w13_d, w2_d, ys_d = [], [], [], []
    for i, n in enumerate(segs):
        xs_d.append(nc.dram_tensor(f"x{i}", [P, D_CH, n], sdt,
                                   kind="ExternalInput"))
        w13_d.append(nc.dram_tensor(f"w13_{i}", [P, F_CH, 2, D_CH, P], sdt,
                                    kind="ExternalInput"))
        w2_d.append(nc.dram_tensor(f"w2_{i}", [P, F_CH, D], sdt,
                                   kind="ExternalInput"))
        ys_d.append(nc.dram_tensor(f"y{i}", [D, n], fp32,
                                   kind="ExternalOutput"))
    # tiny pass-through token so the test harness can chain executions
    tok = nc.dram_tensor("tok", [1, 1], fp32, kind="ExternalInput")
    tokout = nc.dram_tensor("tokout", [1, 1], fp32, kind="ExternalOutput")

    # per-partition SBUF is ~208KB; big tiles are x (2 bufs), mid (1), and
    # w2res (2 x 44KB) - shrink the slab-prefetch pool when segments grow
    xkb = 2 * sum(2 * n * D_CH * (2 if cdt_name != "float32r" else 4) / 1024
                  for n in segs)
    wb = 10 if xkb < 40 else 6
    with tile.TileContext(nc) as tc:
        with tc.tile_pool(name="xpool", bufs=2) as xpool, \
             tc.tile_pool(name="wpool", bufs=wb) as wpool, \
             tc.tile_pool(name="w2pool", bufs=2) as w2pool, \
             tc.tile_pool(name="midpool", bufs=1) as midpool, \
             tc.tile_pool(name="tmp", bufs=2) as tmp, \
             tc.tile_pool(name="ytmp", bufs=3) as ytmp, \
             tc.tile_pool(name="psA", bufs=2, space="PSUM") as psA, \
             tc.tile_pool(name="psB", bufs=2, space="PSUM") as psB, \
             tc.tile_pool(name="psY", bufs=psy_bufs, space="PSUM") as psY:

            def swiglu(i, ntok):
                x_d, w_d, w2d, y_d = xs_d[i], w13_d[i], w2_d[i], ys_d[i]
                hch = _chunks(ntok, h_cap)
                ych = _chunks(ntok, y_cap)
                # activations resident; split the load per d-chunk so the
                # first matmul only waits for its own slice
                xT_sb = xpool.tile([P, D_CH, ntok], sdt, tag=f"x{i}",
                                   name=f"x_{i}")
                for d in range(D_CH):
                    nc.scalar.dma_start(xT_sb[:, d], x_d.ap()[:, d])
                # w2 resident; slices prefetched inside the h-loop so the
                # load spreads across the whole h-phase
                w2_sb = w2pool.tile([P, F_CH, D], sdt, tag="w2res",
                                    name=f"w2_{i}")
                # mid resident [P(f_inner), F_CH, ntok]
                mid_sb = midpool.tile([P, F_CH, ntok], sdt, tag="mid",
                                      name=f"mid_{i}")

                # ---- h-phase: mid[f, t] = silu(h1) * h3 ----
                for fc in range(F_CH):
                    w_sb = wpool.tile([P, 2, D_CH, P], sdt, tag="w13slab",
                                      name=f"w13s_{i}_{fc}")
                    nc.sync.dma_start(w_sb[:], w_d.ap()[:, fc])
                    nc.gpsimd.dma_start(w2_sb[:, fc], w2d.ap()[:, fc])
                    t0 = 0
                    for tn in hch:
                        ps1 = psA.tile([P, 512], fp32, tag="ps1",
                                       name=f"ps1_{i}_{fc}_{t0}")[:, :tn]
                        for d in range(D_CH):
                            nc.tensor.matmul(
                                ps1, w_sb[:, 0, 0 if ldw_probe else d],
                                xT_sb[:, d, t0:t0 + tn],
                                start=(d == 0), stop=(d == D_CH - 1))
                        ps3 = psB.tile([P, 512], fp32, tag="ps3",
                                       name=f"ps3_{i}_{fc}_{t0}")[:, :tn]
                        for d in range(D_CH):
                            nc.tensor.matmul(
                                ps3, w_sb[:, 1, 0 if ldw_probe else d],
                                xT_sb[:, d, t0:t0 + tn],
                                start=(d == 0), stop=(d == D_CH - 1))
                        silu_sb = tmp.tile([P, 512], fp32, tag="silu",
                                           name=f"silu_{i}_{fc}_{t0}")[:, :tn]
                        nc.scalar.activation(silu_sb, ps1,
                                             mybir.ActivationFunctionType.Silu)
                        nc.vector.tensor_tensor(mid_sb[:, fc, t0:t0 + tn],
                                                silu_sb, ps3,
                                                mybir.AluOpType.mult)
                        t0 += tn

                # ---- y-phase (transposed): y[d, t] = sum_f w2[f, d]*mid[f, t]
                # stationary = w2 block [128f x 128d], moving = mid tokens.
                t0 = 0
                for tn in ych:
                    for dt in range(D_CH):
                        psy = psY.tile([P, min(512, y_cap)], fp32, tag="psy",
                                       name=f"psy_{i}_{t0}_{dt}")[:, :tn]
                        for fc in range(F_CH):
                            nc.tensor.matmul(
                                psy,
                                w2_sb[:, 0 if ldw_probe else fc,
                                      dt * P:(dt + 1) * P],
                                mid_sb[:, fc, t0:t0 + tn],
                                start=(fc == 0), stop=(fc == F_CH - 1))
                        y_sb = ytmp.tile([P, min(512, y_cap)], fp32, tag="ysb",
                                         name=f"y_{i}_{t0}_{dt}")[:, :tn]
                        if ycopy_dve:
                            nc.vector.tensor_copy(y_sb, psy)
                        else:
                            nc.scalar.copy(y_sb, psy)
                        nc.scalar.dma_start(
                            y_d.ap()[dt * P:(dt + 1) * P, t0:t0 + tn],
                            y_sb)
                    t0 += tn

            def body():
                for i, n in enumerate(segs):
                    swiglu(i, n)

            if reps == 1:
                body()
            else:
                # staggered_reset avoids the ~2us all-engine barrier per
                # back-edge; hint PE so the >256-inst body's back-edge
                # branch target is prefetched into IRAM
                with tc.For_i(0, reps, 1, staggered_reset=True,
                              hint_engines=(mybir.EngineType.PE,)):
                    body()
            nc.sync.dma_start(tokout.ap(), tok.ap())

    nc.compile()
    _BUILD_CACHE[key] = nc
    return nc


def _sigmoid32(x):
    x = x.astype(np.float32)
    return np.where(x >= 0, 1.0 / (1.0 + np.exp(-x)),
                    np.exp(x) / (1.0 + np.exp(x))).astype(np.float32)


def _np_dt(cdt_name):
    if cdt_name == "bfloat16":
        import ml_dtypes
        return ml_dtypes.bfloat16
    if cdt_name == "float16":
        return np.float16
    return np.float32


def _pack_w13(w1, w3, np_dt):
    # 2x [D, F] -> [P(d_inner), F_CH, 2, D_CH, P(f_inner)]
    def pk(w):
        return w.reshape(D_CH, P, F_CH, P).transpose(1, 2, 0, 3)
    return np.ascontiguousarray(
        np.stack([pk(w1), pk(w3)], axis=2).astype(np_dt))


def _pack_w2(w, np_dt):
    # [F, D] -> [P(f_inner), F_CH, D]
    return np.ascontiguousarray(
        w.reshape(F_CH, P, D).transpose(1, 0, 2).astype(np_dt))


def _pack_xT(x, np_dt, n):
    # [k, D] (k<=n, zero-padded to n) -> [P(d_inner), D_CH, n]
    if x.shape[0] < n:
        x = np.concatenate(
            [x, np.zeros((n - x.shape[0], D), np.float32)], axis=0)
    return np.ascontiguousarray(
        x.reshape(n, D_CH, P).transpose(2, 1, 0).astype(np_dt))


def _solve_slots(counts, caps):
    """Assign each expert a vector a[j] of slots per size-class j (8 slots
    per class, class j holds caps[j] tokens) with sum_j a[j]*caps[j] >=
    count and per-class totals <= E. Returns [a_e vectors] or None."""
    k = len(caps)

    def options(n):
        # pareto set of slot-count vectors covering n tokens
        opts = set()

        def rec(j, vec, cov):
            if cov >= n:
                opts.add(tuple(vec) + (0,) * (k - len(vec)))
                return
            if j == k:
                return
            maxa = min(E, -(-(n - cov) // caps[j]))
            for a in range(maxa + 1):
                vec.append(a)
                rec(j + 1, vec, cov + a * caps[j])
                vec.pop()
        rec(0, [], 0)
        # prune dominated (elementwise >=)
        out = []
        for v in sorted(opts, key=sum):
            if not any(all(u[i] <= v[i] for i in range(k)) and u != v
                       for u in out):
                out.append(v)
        return out

    opts = [options(int(n)) for n in counts]
    order = np.argsort(counts)[::-1]
    assign = [None] * len(counts)

    def dfs(i, used):
        if i == len(order):
            return True
        e = order[i]
        for v in opts[e]:
            nu = tuple(used[j] + v[j] for j in range(k))
            if all(u <= E for u in nu):
                assign[e] = v
                if dfs(i + 1, nu):
                    return True
        assign[e] = None
        return False

    return assign if dfs(0, (0,) * k) else None


def prepare(x_bsD, router_DE, w1_eDF, w3_eDF, w2_eFD, ws1_DF, ws3_DF, ws2_FD,
            cdt_name="float16", segs=SEGS_DEFAULT):
    """Host-side routing + dispatch. Returns (in_maps, aux) for the SPMD run."""
    np_dt = _np_dt(cdt_name)

    x = np.ascontiguousarray(np.asarray(x_bsD, np.float32).reshape(A, D))
    scores = x @ np.asarray(router_DE, np.float32)          # [A, E]
    top1 = np.argmax(scores, axis=1)                        # [A]
    gate = _sigmoid32(scores[np.arange(A), top1])           # [A]

    idx_e = [np.nonzero(top1 == e)[0] for e in range(E)]
    counts = np.array([len(i) for i in idx_e])

    caps = list(segs[1:])
    assign = _solve_slots(counts, caps)
    while assign is None:
        caps = [c + 16 for c in caps]
        assign = _solve_slots(counts, caps)
    segs = (T, *caps)

    # Distribute each expert's tokens into its slots (largest class first).
    slots = [[] for _ in caps]   # per class: list of (expert, token_idx)
    for e in range(E):
        pos = 0
        for j in range(len(caps)):
            for _ in range(assign[e][j]):
                take = min(caps[j], counts[e] - pos)
                slots[j].append((e, idx_e[e][pos:pos + take]))
                pos += take
        assert pos >= counts[e]
    for j in range(len(caps)):
        while len(slots[j]) < E:
            slots[j].append((0, np.zeros(0, np.int64)))

    w13p = {}
    w2p = {}
    for e in range(E):
        w13p[e] = _pack_w13(np.asarray(w1_eDF[e], np.float32),
                            np.asarray(w3_eDF[e], np.float32), np_dt)
        w2p[e] = _pack_w2(np.asarray(w2_eFD[e], np.float32), np_dt)
    ws13p = _pack_w13(np.asarray(ws1_DF, np.float32),
                      np.asarray(ws3_DF, np.float32), np_dt)
    ws2p = _pack_w2(np.asarray(ws2_FD, np.float32), np_dt)

    in_maps = []
    for c in range(E):
        m = {
            "x0": _pack_xT(x[c * T:(c + 1) * T], np_dt, T),
            "w13_0": ws13p, "w2_0": ws2p,
            "tok": np.zeros((1, 1), np.float32),
        }
        for j in range(len(caps)):
            e, idx = slots[j][c]
            m[f"x{j + 1}"] = _pack_xT(gate[idx, None] * x[idx], np_dt,
                                      caps[j])
            m[f"w13_{j + 1}"] = w13p[e]
            m[f"w2_{j + 1}"] = w2p[e]
        in_maps.append(m)
    return in_maps, (slots, segs)


def combine(results, aux):
    """Merge per-core outputs into the full [B, S, D] output."""
    slots, segs = aux
    out = np.empty((A, D), np.float32)
    for c in range(E):
        out[c * T:(c + 1) * T] = results[c]["y0"].T
    for c in range(E):
        for j in range(len(segs) - 1):
            _, idx = slots[j][c]
            if len(idx):
                out[idx] += results[c][f"y{j + 1}"][:, :len(idx)].T
    return out.reshape(B, S, D)


def kernel(x_bsD, router_DE, w1_eDF, w3_eDF, w2_eFD, ws1_DF, ws3_DF, ws2_FD,
           cdt_name="float16", segs=SEGS_DEFAULT):
    in_maps, aux = prepare(x_bsD, router_DE, w1_eDF, w3_eDF, w2_eFD,
                           ws1_DF, ws3_DF, ws2_FD, cdt_name=cdt_name,
                           segs=segs)
    nc = _build(cdt_name, aux[1])
    res = bass_utils.run_bass_kernel_spmd(nc, in_maps, core_ids=list(range(E)))
    return combine(res.results, aux)

